# revision 17
# baseline (speedup 1.0000x reference)
"""Trainium2 Bass kernel for the Air3D CNF ROM model (nn_Air3DCNFROM).

Model: out[b] = lx(x_b) + tau_b * u_b where
  lx = sqrt(x0^2 + x1^2) - 0.25
  u  = decoder MLP([fourier(x), alpha(tau)])  (106 -> 512 -> 512 -> 512 -> 1, tanh)
  alpha(tau) = linear interp at tau of a latent RK4 trajectory traj[101, 10].

Key structural facts used:
  * alpha0 is zeros and the pnode dynamics depend only on (a, t), so the RK4
    latent trajectory is IDENTICAL for every batch row. It is a [101, 10]
    table computed once on the host (float32, mirroring the reference's
    fixed-step RK4) from the tiny pnode weights.
  * alpha(tau) = traj^T @ hatw(tau) where hatw[s, b] = relu(1 - |tau_b/dtau - s|)
    (linear-interpolation hat weights) -> one [101,10]x[101,512] matmul/tile.
  * fourier features: sin/cos(2*pi*f_j*x_i) computed with explicit range
    reduction (r = y - round(y), y in turns) because the ACT Sin LUT is
    garbage outside a few periods.

Distribution: pure data parallel over 8 NeuronCores (batch 65536 -> 8 x 8192).

Matmuls run in float32r (full-rate fp32 path, ~2^-14 effective operand
precision). ACT writes to float32r tiles are ~4x slower on TRN2, so the
activation tiles are double-buffered raw SBUF buffers aliased under both
float32 (ACT/DVE writers) and float32r (PE reader) handles; the cross-dtype
RAW/WAR dependencies that TileContext cannot see (it keys on (tensor, range))
are added explicitly with add_dep_helper.
"""
import numpy as np

import concourse.bass as bass
import concourse.tile as tile
from concourse import bacc, mybir
from concourse.bass_utils import run_bass_kernel_spmd
from concourse.tile import add_dep_helper

F32 = mybir.dt.float32
F32R = mybir.dt.float32r
I32 = mybir.dt.int32
AF = mybir.ActivationFunctionType
ALU = mybir.AluOpType

N_CORES = 8
B = 65536
B_SHARD = B // N_CORES
NT = 512  # batch tile (psum free dim)
LAT = 10
STEPS = 101
DTAU = np.float32(0.01)
RADIUS = 0.25
N_FREQS = 16
MAX_FREQ = 10.0
PI2 = float(2.0 * np.pi)


def _host_traj(pn_w0, pn_b0, pn_w1, pn_b1, pn_w2, pn_b2):
    """RK4 scan of the pnode ODE for a single zero-initialized latent,
    mirroring the reference's float32 arithmetic."""
    f32 = np.float32
    half_dtau = f32(0.5) * DTAU
    dtau6 = f32(0.01 / 6.0)
    two = f32(2.0)
    ts = np.linspace(0.0, 1.0, STEPS, dtype=np.float32)

    def f(t, a):
        inp = np.concatenate([a, np.full((1, 1), t, np.float32)], axis=1)
        h = np.tanh(inp @ pn_w0 + pn_b0)
        h = np.tanh(h @ pn_w1 + pn_b1)
        return h @ pn_w2 + pn_b2

    a = np.zeros((1, LAT), np.float32)
    traj = np.empty((STEPS, LAT), np.float32)
    traj[0] = a
    for i in range(STEPS - 1):
        t = ts[i]
        k1 = f(t, a)
        k2 = f(t + half_dtau, a + half_dtau * k1)
        k3 = f(t + half_dtau, a + half_dtau * k2)
        k4 = f(t + DTAU, a + DTAU * k3)
        a = a + dtau6 * (k1 + two * k2 + two * k3 + k4)
        traj[i + 1] = a
    return traj


def build_kernel(b_shard: int, b3_val: float, detect_races: bool = True,
                 use_alias: bool = True):
    """Build the single-core Bass program (SPMD across cores).

    use_alias=False replaces each f32/f32r aliased buffer pair with a single
    f32r tensor (CoreSim's memory model rejects aliased SBUF reads); the
    manual dependency edges are still emitted, so the sim validates both the
    math and that the edge graph is deadlock-free.
    """
    n_tiles = b_shard // NT
    assert b_shard % NT == 0

    nc = bacc.Bacc("TRN2", target_bir_lowering=False, debug=False,
                   detect_race_conditions=detect_races)

    # ---- DRAM I/O
    d_bc96 = nc.dram_tensor("bc96", [96, b_shard], F32, kind="ExternalInput").ap()
    d_t101 = nc.dram_tensor("t101", [STEPS, b_shard], F32, kind="ExternalInput").ap()
    d_xnat = nc.dram_tensor("xnat", [b_shard, 3], F32, kind="ExternalInput").ap()
    d_taun = nc.dram_tensor("taun", [b_shard], F32, kind="ExternalInput").ap()
    d_w0 = nc.dram_tensor("w0", [106, 512], F32R, kind="ExternalInput").ap()
    d_w1 = nc.dram_tensor("w1", [512, 512], F32R, kind="ExternalInput").ap()
    d_w2 = nc.dram_tensor("w2", [512, 512], F32R, kind="ExternalInput").ap()
    d_w3c = nc.dram_tensor("w3c", [128, 4], F32R, kind="ExternalInput").ap()
    d_b0c = nc.dram_tensor("b0c", [128, 4], F32, kind="ExternalInput").ap()
    d_b1c = nc.dram_tensor("b1c", [128, 4], F32, kind="ExternalInput").ap()
    d_b2c = nc.dram_tensor("b2c", [128, 4], F32, kind="ExternalInput").ap()
    d_traj = nc.dram_tensor("trajc", [STEPS, LAT], F32R, kind="ExternalInput").ap()
    d_iota = nc.dram_tensor("iota", [STEPS, 1], F32, kind="ExternalInput").ap()
    d_f96 = nc.dram_tensor("f96", [96, 1], F32, kind="ExternalInput").ap()
    d_ph96 = nc.dram_tensor("ph96", [96, 1], F32, kind="ExternalInput").ap()
    d_out = nc.dram_tensor("out", [b_shard], F32, kind="ExternalOutput").ap()

    # ---- aliased activation buffers (f32 written by ACT/DVE, f32r read by PE)
    alias_map: dict = {}

    def alias_pair(name, cols):
        if not use_alias:
            t = nc.alloc_sbuf_tensor(f"{name}_f32r", [128, cols], F32R)
            return t, t
        t32 = nc.alloc_sbuf_tensor(f"{name}_f32", [128, cols], F32)
        addr = nc.lookup_mloc(t32).addr
        t32r = nc.alloc_sbuf_tensor_at(f"{name}_f32r", [128, cols], F32R, offset=addr)
        alias_map[t32r.name] = t32.name
        return t32, t32r

    h0 = [alias_pair(f"h0_{s}", NT) for s in range(2)]
    h1 = [alias_pair(f"h1_{s}", 4 * NT) for s in range(2)]
    h2 = [alias_pair(f"h2_{s}", 4 * NT) for s in range(2)]
    h3 = [alias_pair(f"h3_{s}", 4 * NT) for s in range(2)]

    last_readers: dict = {}

    def link(key, writers, readers):
        """Manual cross-alias dependencies: WAR vs previous round's readers,
        RAW from this round's writers to this round's readers."""
        for w in writers:
            for r in last_readers.get(key, ()):
                add_dep_helper(w.ins, r.ins, reason="alias-WAR")
        for r in readers:
            for w in writers:
                add_dep_helper(r.ins, w.ins, reason="alias-RAW")
        last_readers[key] = readers

    with tile.TileContext(nc) as tc:
        with tc.tile_pool(name="res", bufs=1) as res, \
             tc.tile_pool(name="tmp", bufs=2) as tmp, \
             tc.tile_pool(name="ps", bufs=8, space="PSUM") as ps:

            # ---- resident tensors
            bc96_sb = res.tile([96, b_shard], F32, name="bc96_sb")
            t101_sb = res.tile([STEPS, b_shard], F32, name="t101_sb")
            for t in range(n_tiles):
                cs = bass.ts(t, NT)
                nc.sync.dma_start(bc96_sb[:, cs], d_bc96[:, cs])
                nc.sync.dma_start(t101_sb[:, cs], d_t101[:, cs])

            w0_sb = res.tile([106, 512], F32R, name="w0_sb")
            nc.sync.dma_start(w0_sb[:], d_w0)
            w1_sb = [res.tile([128, 512], F32R, name=f"w1_sb{k}") for k in range(4)]
            w2_sb = [res.tile([128, 512], F32R, name=f"w2_sb{k}") for k in range(4)]
            for k in range(4):
                nc.sync.dma_start(w1_sb[k][:], d_w1[bass.ts(k, 128), :])
                nc.sync.dma_start(w2_sb[k][:], d_w2[bass.ts(k, 128), :])
            w3_sb = res.tile([128, 4], F32R, name="w3_sb")
            nc.sync.dma_start(w3_sb[:], d_w3c)
            b0_sb = res.tile([128, 4], F32, name="b0_sb")
            nc.sync.dma_start(b0_sb[:], d_b0c)
            b1_sb = res.tile([128, 4], F32, name="b1_sb")
            nc.sync.dma_start(b1_sb[:], d_b1c)
            b2_sb = res.tile([128, 4], F32, name="b2_sb")
            nc.sync.dma_start(b2_sb[:], d_b2c)
            traj_sb = res.tile([STEPS, LAT], F32R, name="traj_sb")
            nc.sync.dma_start(traj_sb[:], d_traj)
            iota_sb = res.tile([STEPS, 1], F32, name="iota_sb")
            nc.sync.dma_start(iota_sb[:], d_iota)
            f96_sb = res.tile([96, 1], F32, name="f96_sb")
            nc.sync.dma_start(f96_sb[:], d_f96)
            ph96_sb = res.tile([96, 1], F32, name="ph96_sb")
            nc.sync.dma_start(ph96_sb[:], d_ph96)
            ident = res.tile([1, 1], F32, name="ident")
            nc.vector.memset(ident[:], 1.0)
            # u gathered column-wise via PE transpose; u_sb[p, 4t+c] holds
            # sample b = 512*t + 128*c + p
            u_sb = res.tile([128, b_shard // 128], F32, name="u_sb")

            # ---- main loop over batch tiles
            for t in range(n_tiles):
                s = t % 2
                cs = bass.ts(t, NT)
                h0_32, h0_r = h0[s]
                h1_32, h1_r = h1[s]
                h2_32, h2_r = h2[s]
                h3_32, h3_r = h3[s]

                # fourier features in turn units: y = f*x + phase + 128 (>0);
                # r = y - int(y) folded into [-0.5, 0.5] (portable for both
                # truncating and round-to-nearest f32->i32 conversion), then
                # sin(2*pi*r) on ACT.
                proj = tmp.tile([96, NT], F32, tag="proj")
                nc.vector.tensor_scalar(proj[:], bc96_sb[:, cs], f96_sb[:],
                                        ph96_sb[:], op0=ALU.mult, op1=ALU.add)
                ri = tmp.tile([96, NT], I32, tag="ri")
                nc.vector.tensor_copy(ri[:], proj[:])
                rf = tmp.tile([96, NT], F32, tag="rf")
                nc.vector.tensor_copy(rf[:], ri[:])
                rr = tmp.tile([96, NT], F32, tag="rr")
                nc.vector.tensor_sub(rr[:], proj[:], rf[:])
                msk = tmp.tile([96, NT], F32, tag="msk")
                nc.vector.tensor_scalar(msk[:], rr[:], 0.5, None, op0=ALU.is_gt)
                rrf = tmp.tile([96, NT], F32, tag="rrf")
                nc.vector.tensor_sub(rrf[:], rr[:], msk[:])
                sin_i = nc.scalar.activation(h0_32.ap()[0:96, :], rrf[:], AF.Sin,
                                             scale=PI2)

                # interpolation hat weights: W = relu(1 - |t101 - s|)
                hd = tmp.tile([STEPS, NT], F32, tag="hd")
                nc.vector.tensor_scalar(hd[:], t101_sb[:, cs], iota_sb[:], None,
                                        op0=ALU.subtract)
                hn = tmp.tile([STEPS, NT], F32, tag="hn")
                nc.vector.tensor_scalar(hn[:], hd[:], -1.0, None, op0=ALU.mult)
                ha = tmp.tile([STEPS, NT], F32, tag="ha")
                nc.vector.tensor_tensor(ha[:], hd[:], hn[:], op=ALU.max)
                hm = tmp.tile([STEPS, NT], F32, tag="hm")
                nc.vector.tensor_scalar(hm[:], ha[:], -1.0, 1.0,
                                        op0=ALU.mult, op1=ALU.add)
                hw = tmp.tile([STEPS, NT], F32R, tag="hw")
                nc.vector.tensor_scalar(hw[:], hm[:], 0.0, None, op0=ALU.max)

                # alpha = traj^T @ W -> h0 rows 96:106
                p_al = ps.tile([128, NT], F32, tag="mm", name=f"p_al_{t}")
                nc.tensor.matmul(p_al[0:LAT, :], traj_sb[:], hw[:],
                                 start=True, stop=True)
                acopy_i = nc.vector.tensor_copy(h0_32.ap()[96:96 + LAT, :],
                                                p_al[0:LAT, :])

                # decoder layer 1: [106 -> 512]
                l1_mms = []
                p_l1 = [ps.tile([128, NT], F32, tag="mm", name=f"p_l1_{t}_{m}") for m in range(4)]
                for m in range(4):
                    mm = nc.tensor.matmul(p_l1[m][:], w0_sb[:, bass.ts(m, 128)],
                                          h0_r.ap()[0:106, :], start=True, stop=True)
                    l1_mms.append(mm)
                link(("h0", s), [sin_i, acopy_i], l1_mms)
                tanh1 = []
                for m in range(4):
                    a = nc.scalar.activation(h1_32.ap()[:, bass.ts(m, NT)],
                                             p_l1[m][:], AF.Tanh,
                                             bias=b0_sb[:, m:m + 1])
                    tanh1.append(a)

                # layer 2: [512 -> 512]
                l2_mms = [[] for _ in range(4)]  # readers by k-chunk
                p_l2 = [ps.tile([128, NT], F32, tag="mm", name=f"p_l2_{t}_{m}") for m in range(4)]
                for m in range(4):
                    for k in range(4):
                        mm = nc.tensor.matmul(p_l2[m][:],
                                              w1_sb[k][:, bass.ts(m, 128)],
                                              h1_r.ap()[:, bass.ts(k, NT)],
                                              start=(k == 0), stop=(k == 3))
                        l2_mms[k].append(mm)
                for k in range(4):
                    link(("h1", s, k), [tanh1[k]], l2_mms[k])
                tanh2 = []
                for m in range(4):
                    a = nc.scalar.activation(h2_32.ap()[:, bass.ts(m, NT)],
                                             p_l2[m][:], AF.Tanh,
                                             bias=b1_sb[:, m:m + 1])
                    tanh2.append(a)

                # layer 3: [512 -> 512]
                l3_mms = [[] for _ in range(4)]
                p_l3 = [ps.tile([128, NT], F32, tag="mm", name=f"p_l3_{t}_{m}") for m in range(4)]
                for m in range(4):
                    for k in range(4):
                        mm = nc.tensor.matmul(p_l3[m][:],
                                              w2_sb[k][:, bass.ts(m, 128)],
                                              h2_r.ap()[:, bass.ts(k, NT)],
                                              start=(k == 0), stop=(k == 3))
                        l3_mms[k].append(mm)
                for k in range(4):
                    link(("h2", s, k), [tanh2[k]], l3_mms[k])
                tanh3 = []
                for m in range(4):
                    a = nc.scalar.activation(h3_32.ap()[:, bass.ts(m, NT)],
                                             p_l3[m][:], AF.Tanh,
                                             bias=b2_sb[:, m:m + 1])
                    tanh3.append(a)

                # layer 4: [512 -> 1], u = w3^T h3 + b3
                p_u = ps.tile([128, NT], F32, tag="mm", name=f"p_u_{t}")
                for k in range(4):
                    mm = nc.tensor.matmul(p_u[0:1, :], w3_sb[:, k:k + 1],
                                          h3_r.ap()[:, bass.ts(k, NT)],
                                          start=(k == 0), stop=(k == 3))
                    link(("h3", s, k), [tanh3[k]], [mm])
                strip = tmp.tile([1, NT], F32, tag="strip")
                nc.vector.tensor_scalar(strip[:], p_u[0:1, :], float(b3_val),
                                        None, op0=ALU.add)
                # repartition [1, 512] -> [128, 4] via PE transpose
                p_t = ps.tile([128, NT], F32, tag="mm", name=f"p_t_{t}")
                for c in range(4):
                    nc.tensor.transpose(p_t[:, c:c + 1],
                                        strip[0:1, bass.ts(c, 128)], ident[:])
                nc.vector.tensor_copy(u_sb[:, bass.ts(t, 4)], p_t[:, 0:4])

            # ---- final combine on [128, b_shard/128]: out = lx + tau*u
            # column m = 4t+c of u_sb holds samples b = 512t + 128c + p, so
            # x/tau/out use the matching "(t c p)" layout.
            q = b_shard // 128
            x_sb = tmp.tile([128, 3 * q], F32, tag="x_sb", bufs=1)
            nc.sync.dma_start(
                x_sb[:], d_xnat.rearrange("(t c p) v -> p t c v", p=128, c=4))
            tau_sb = tmp.tile([128, q], F32, tag="tau_sb", bufs=1)
            nc.sync.dma_start(
                tau_sb[:], d_taun.rearrange("(t c p) -> p t c", p=128, c=4))

            xv = x_sb[:].rearrange("p (q c) -> p c q", c=3)
            t1 = tmp.tile([128, q], F32, tag="t1", bufs=1)
            nc.vector.tensor_tensor(t1[:], xv[:, 0:1, :], xv[:, 0:1, :],
                                    op=ALU.mult)
            t2 = tmp.tile([128, q], F32, tag="t2", bufs=1)
            nc.vector.tensor_tensor(t2[:], xv[:, 1:2, :], xv[:, 1:2, :],
                                    op=ALU.mult)
            ss = tmp.tile([128, q], F32, tag="ss", bufs=1)
            nc.vector.tensor_add(ss[:], t1[:], t2[:])
            sq = tmp.tile([128, q], F32, tag="sq", bufs=1)
            nc.scalar.activation(sq[:], ss[:], AF.Sqrt)
            mu = tmp.tile([128, q], F32, tag="mu", bufs=1)
            nc.vector.tensor_tensor(mu[:], tau_sb[:], u_sb[:], op=ALU.mult)
            ad = tmp.tile([128, q], F32, tag="ad", bufs=1)
            nc.vector.tensor_tensor(ad[:], mu[:], sq[:], op=ALU.add)
            fin = tmp.tile([128, q], F32, tag="fin", bufs=1)
            nc.vector.tensor_scalar(fin[:], ad[:], -float(RADIUS), None,
                                    op0=ALU.add)
            nc.sync.dma_start(
                d_out.rearrange("(t c p) -> p t c", p=128, c=4), fin[:])

    nc.finalize()
    nc._air3d_alias_map = alias_map
    return nc


def _prepare_core_inputs(x, tau, dec_w0, dec_b0, dec_w1, dec_b1, dec_w2, dec_b2,
                         dec_w3, dec_b3, traj):
    """Host-side sharding + layout prep. Returns list of per-core in_maps."""
    freqs = np.linspace(1.0, MAX_FREQ, N_FREQS, dtype=np.float32)
    # fourier slot layout: p = i*32 + j (sin), p = i*32 + 16 + j (cos)
    coord_of_slot = np.repeat(np.arange(3), 32)
    f96 = np.tile(np.concatenate([freqs, freqs]), 3).astype(np.float32)
    ph96 = np.tile(np.concatenate([np.zeros(16, np.float32),
                                   np.full(16, 0.25, np.float32)]), 3) + np.float32(128.0)

    iota = np.arange(STEPS, dtype=np.float32).reshape(STEPS, 1)
    w3c = np.ascontiguousarray(dec_w3.reshape(4, 128).T)
    b0c = np.ascontiguousarray(dec_b0.reshape(4, 128).T)
    b1c = np.ascontiguousarray(dec_b1.reshape(4, 128).T)
    b2c = np.ascontiguousarray(dec_b2.reshape(4, 128).T)

    in_maps = []
    for c in range(N_CORES):
        sl = slice(c * B_SHARD, (c + 1) * B_SHARD)
        xs = np.ascontiguousarray(x[sl])
        taus = np.ascontiguousarray(tau[sl])
        tau100 = taus / DTAU
        xT = xs.T  # [3, B_SHARD]
        bc96 = np.ascontiguousarray(xT[coord_of_slot])  # [96, B_SHARD]
        t101 = np.ascontiguousarray(
            np.broadcast_to(tau100[None, :], (STEPS, B_SHARD)))
        in_maps.append({
            "bc96": bc96, "t101": t101, "xnat": xs, "taun": taus,
            "w0": np.ascontiguousarray(dec_w0),
            "w1": np.ascontiguousarray(dec_w1),
            "w2": np.ascontiguousarray(dec_w2),
            "w3c": w3c, "b0c": b0c, "b1c": b1c, "b2c": b2c,
            "trajc": traj, "iota": iota,
            "f96": f96.reshape(96, 1), "ph96": ph96.reshape(96, 1),
        })
    return in_maps


def run(inputs: dict, trace: bool = False):
    """Build, run on 8 cores, gather. Returns (out, BassKernelResults)."""
    traj = _host_traj(inputs["pn_w0"], inputs["pn_b0"], inputs["pn_w1"],
                      inputs["pn_b1"], inputs["pn_w2"], inputs["pn_b2"])
    nc = build_kernel(B_SHARD, float(np.asarray(inputs["dec_b3"]).reshape(-1)[0]))
    in_maps = _prepare_core_inputs(
        np.asarray(inputs["x"], np.float32), np.asarray(inputs["tau"], np.float32),
        np.asarray(inputs["dec_w0"], np.float32), np.asarray(inputs["dec_b0"], np.float32),
        np.asarray(inputs["dec_w1"], np.float32), np.asarray(inputs["dec_b1"], np.float32),
        np.asarray(inputs["dec_w2"], np.float32), np.asarray(inputs["dec_b2"], np.float32),
        np.asarray(inputs["dec_w3"], np.float32), np.asarray(inputs["dec_b3"], np.float32),
        traj)
    res = run_bass_kernel_spmd(nc, in_maps, list(range(N_CORES)), trace=trace)
    out = np.concatenate([res.results[c]["out"] for c in range(N_CORES)])
    return out, res


def kernel(**inputs) -> np.ndarray:
    out, _ = run(inputs, trace=False)
    return out
